# revision 2
# baseline (speedup 1.0000x reference)
"""BilinearInteraction (field_interaction) on 8 TRN2 NeuronCores.

  out[b,f,d] = emb[b,f,d] * sum_{g!=f, e} W[f,g,d,e] * emb[b,g,e]

Strategy (data-parallel, per sharding hint):
  - Host: fold the f!=g mask into W and permute it to a single GEMM matrix
    w2[g*32+e, f*32+d]; flatten embeddings to x[B, 1280]; shard batch over
    8 cores (2048 rows each); replicate w2.
  - Precision: x and w2 ship as fp16 (halves HBM traffic; PSUM accumulates
    fp32; rel err ~5e-4, far inside the gate).
  - Device (per core): out = x * (x @ w2): fp16 TensorEngine matmuls at
    1 cyc/row. x row-blocks are transposed on-chip with PE transpose-mode
    matmuls whose 16-bit payloads are *labeled* bf16 (fp16 transpose-mode
    is broken on TRN2 silicon; x1.0-in-bf16 is bit-exact), packed
    4-per-PSUM-bank and drained to SBUF by the otherwise-idle Scalar
    engine, then bitcast back to fp16 as the matmul stationary operand.
    w2 stays SBUF-resident in per-k-tile tiles (one coalesced DMA each --
    the startup ramp is bound by ~0.7us/dma_start sequencer issue time, so
    few big DMAs beat many small ones); transposes of batch-tile bt+1 are
    software-pipelined into the matmul stream of batch-tile bt; DVE does
    the final elementwise multiply out of PSUM. The first two batch tiles
    consume W k-chunks interleaved so the HBM-paced startup ramp keeps the
    TensorEngine fed.
"""

from contextlib import ExitStack

import numpy as np

BATCH = 16384
NUM_FIELDS = 40
EMBED_DIM = 32
N_CORES = 8

B_LOCAL = BATCH // N_CORES   # 2048
K = NUM_FIELDS * EMBED_DIM   # 1280
P = 128
NBT = B_LOCAL // P           # 16
NKT = K // P                 # 10
N_CHUNKS = [(0, 512), (512, 512), (1024, 256)]
TR_GROUPS = [(0, 4), (4, 4), (8, 2)]

_NC_CACHE = {}


def _build_kernel():
    import concourse.bacc as bacc
    import concourse.mybir as mybir
    import concourse.tile as tile

    F32 = mybir.dt.float32
    FP16 = mybir.dt.float16
    BF16 = mybir.dt.bfloat16
    N = K

    nc = bacc.Bacc("TRN2", target_bir_lowering=False, debug=False, num_devices=N_CORES)

    x_d = nc.declare_dram_parameter("x", [B_LOCAL, K], FP16, isOutput=False)
    w_d = nc.declare_dram_parameter("w2", [K, N], FP16, isOutput=False)
    i_d = nc.declare_dram_parameter("ident", [P, P], BF16, isOutput=False)
    o_d = nc.declare_dram_parameter("out", [B_LOCAL, N], F32, isOutput=True)

    with tile.TileContext(nc) as tc, ExitStack() as ctx:
        wpool = ctx.enter_context(tc.tile_pool(name="w", bufs=1))
        cpool = ctx.enter_context(tc.tile_pool(name="const", bufs=1))
        xpool = ctx.enter_context(tc.tile_pool(name="x", bufs=4))
        xtpool = ctx.enter_context(tc.tile_pool(name="xt", bufs=3))
        opool = ctx.enter_context(tc.tile_pool(name="o", bufs=4))
        trps = ctx.enter_context(tc.tile_pool(name="trps", bufs=2, space="PSUM"))
        accps = ctx.enter_context(tc.tile_pool(name="accps", bufs=2, space="PSUM"))

        ident = cpool.tile([P, P], BF16)
        nc.sync.dma_start(ident[:], i_d[:])

        x_tiles = {}

        def load_x(bt):
            t = xpool.tile([P, K], FP16, name=f"x{bt}", tag="x_sb")
            nc.sync.dma_start(t[:], x_d[bt * P:(bt + 1) * P, :])
            x_tiles[bt] = t

        load_x(0)
        load_x(1)

        w_sb = []
        for kt in range(NKT):
            wt = wpool.tile([P, N], FP16, name=f"w{kt}", tag=f"w{kt}")
            nc.sync.dma_start(wt[:], w_d[kt * P:(kt + 1) * P, :])
            w_sb.append(wt)

        xt_tiles = {}

        def emit_tr_group(bt, g):
            kt0, cnt = TR_GROUPS[g]
            x_sb = x_tiles[bt]
            tr = trps.tile([P, 512], BF16, name=f"tr{bt}_{g}", tag="tr")
            for i in range(cnt):
                kt = kt0 + i
                nc.tensor.transpose(tr[:, i * P:(i + 1) * P],
                                    x_sb[:, kt * P:(kt + 1) * P].bitcast(BF16),
                                    ident[:])
            nc.scalar.copy(xt_tiles[bt][:, kt0 * P:(kt0 + cnt) * P],
                           tr[:, 0:cnt * P])

        for b in (0, 1):
            xt_tiles[b] = xtpool.tile([P, K], BF16, name=f"xt{b}", tag="xt")
        for g in range(3):
            emit_tr_group(0, g)
            emit_tr_group(1, g)

        accs = {}

        def alloc_acc(bt):
            accs[bt] = [accps.tile([P, sz], F32, name=f"acc{j}_{bt}",
                                   tag=f"acc{j}")
                        for j, (_, sz) in enumerate(N_CHUNKS)]

        def emit_mms(bt, kt):
            lhsT = xt_tiles[bt][:, kt * P:(kt + 1) * P].bitcast(FP16)
            for j, (n0, sz) in enumerate(N_CHUNKS):
                nc.tensor.matmul(accs[bt][j][:], lhsT, w_sb[kt][:, n0:n0 + sz],
                                 start=(kt == 0), stop=(kt == NKT - 1))

        def emit_epilogue(bt, per_chunk_out):
            o_sb = opool.tile([P, N], F32, name=f"o{bt}", tag="o")
            for j, (n0, sz) in enumerate(N_CHUNKS):
                nc.vector.tensor_mul(o_sb[:, n0:n0 + sz],
                                     x_tiles[bt][:, n0:n0 + sz],
                                     accs[bt][j][:])
                if per_chunk_out:
                    nc.sync.dma_start(o_d[bt * P:(bt + 1) * P, n0:n0 + sz],
                                      o_sb[:, n0:n0 + sz])
            if not per_chunk_out:
                nc.sync.dma_start(o_d[bt * P:(bt + 1) * P, :], o_sb[:])

        alloc_acc(0)
        alloc_acc(1)
        for kt in range(NKT):
            emit_mms(0, kt)
            emit_mms(1, kt)
            if kt == 3:
                load_x(2)
                xt_tiles[2] = xtpool.tile([P, K], BF16, name="xt2", tag="xt")
            elif kt in (6, 7, 8):
                emit_tr_group(2, kt - 6)
        emit_epilogue(0, False)
        emit_epilogue(1, False)

        for bt in range(2, NBT):
            alloc_acc(bt)
            last = bt == NBT - 1
            for kt in range(NKT):
                emit_mms(bt, kt)
                if bt + 1 < NBT:
                    if kt == 3:
                        load_x(bt + 1)
                        xt_tiles[bt + 1] = xtpool.tile([P, K], BF16,
                                                       name=f"xt{bt+1}",
                                                       tag="xt")
                    elif kt in (6, 7, 8):
                        emit_tr_group(bt + 1, kt - 6)
            emit_epilogue(bt, last)
            if bt > 2:
                del x_tiles[bt - 1], xt_tiles[bt - 1], accs[bt - 1]

    nc.compile()
    return nc


def _get_nc():
    if "nc" not in _NC_CACHE:
        _NC_CACHE["nc"] = _build_kernel()
    return _NC_CACHE["nc"]


def _ensure_axon_hooks():
    """bass_utils imports antenv.axon_hooks when BASS_TRACE is set; provide a
    no-op registry if the environment lacks it so tracing degrades gracefully
    instead of crashing."""
    try:
        import antenv.axon_hooks  # noqa: F401
    except ImportError:
        import sys
        import types

        m = types.ModuleType("antenv.axon_hooks")
        m._HOOK = None
        m.set_axon_ntff_profile_hook = lambda h: setattr(m, "_HOOK", h)
        m.get_axon_ntff_profile_hook = lambda: m._HOOK
        sys.modules["antenv.axon_hooks"] = m
        try:
            from trn_agent_boot.trn_boot import _ntff_profile_via_ctypes

            m._HOOK = _ntff_profile_via_ctypes("/opt/axon/libaxon_pjrt.so")
        except Exception:
            pass


def kernel(embeddings: np.ndarray, bilinear_W: np.ndarray) -> np.ndarray:
    _ensure_axon_hooks()
    from concourse.bass_utils import run_bass_kernel_spmd

    embeddings = np.ascontiguousarray(np.asarray(embeddings, dtype=np.float32))
    bilinear_W = np.ascontiguousarray(np.asarray(bilinear_W, dtype=np.float32))
    F, D = NUM_FIELDS, EMBED_DIM

    # fold the f!=g mask into W and permute to the GEMM layout:
    # w2[g*D+e, f*D+d] = W[f,g,d,e] * (f != g)
    mask = (1.0 - np.eye(F, dtype=np.float32))[:, :, None, None]
    w2 = np.ascontiguousarray(
        (bilinear_W * mask).transpose(1, 3, 0, 2).reshape(F * D, F * D))

    import ml_dtypes

    x = embeddings.reshape(BATCH, F * D).astype(np.float16)
    shards = np.split(x, N_CORES, axis=0)
    ident = np.eye(P, dtype=np.float32).astype(ml_dtypes.bfloat16)
    w2_h = w2.astype(np.float16)
    in_maps = [{"x": np.ascontiguousarray(s), "w2": w2_h, "ident": ident}
               for s in shards]

    nc = _get_nc()
    res = run_bass_kernel_spmd(nc, in_maps, list(range(N_CORES)))
    out = np.concatenate([res.results[i]["out"] for i in range(N_CORES)],
                         axis=0)
    return out.reshape(BATCH, F, D).astype(np.float32, copy=False)



# revision 4
# speedup vs baseline: 1.1377x; 1.1377x over previous
"""BilinearInteraction (field_interaction) on 8 TRN2 NeuronCores.

  out[b,f,d] = emb[b,f,d] * sum_{g!=f, e} W[f,g,d,e] * emb[b,g,e]

Design (see v4-v6 history): host pre-transposes x per core to xT[1280,2048]
fp16 and folds the mask into w2[1280,1280] fp16; device computes
outT = xT * (w2-blocks^T @ xT) with w2 128x128 blocks stationary and xT
k-tiles moving; per-chunk PSUM tiles (4 tags x bufs=2 x 1 bank = all 8
banks); k-contiguous chunk processing so epilogue drains hide under the
next chunk's matmuls; fp16 outputs; host un-transposes.

v6 → v8: the ramp-critical DMA stream is only (w2 cols 0:256 = ramp
n-tiles 0,1) + xT k-tiles = 5.76MB, so pairs arrive every ~1.4us against
~1.7us of matmul work per pair — the PE stays ahead of issue jitter.
w2 cols 256:1280 stream in behind the ramp (first needed ~7us later).
"""

from contextlib import ExitStack

import numpy as np

BATCH = 16384
NUM_FIELDS = 40
EMBED_DIM = 32
N_CORES = 8

B_LOCAL = BATCH // N_CORES   # 2048
K = NUM_FIELDS * EMBED_DIM   # 1280
P = 128
NKT = K // P                 # 10 k-tiles (also 10 n-tiles)
NCH = B_LOCAL // 512         # 4 moving chunks of 512
WA = 256                     # w2 column split: [0:256] (ramp n-tiles 0,1)
N_WARM = 20                  # scratch matmuls to warm the PE clock

_NC_CACHE = {}


def _build_kernel():
    import concourse.bacc as bacc
    import concourse.mybir as mybir
    import concourse.tile as tile

    F32 = mybir.dt.float32
    FP16 = mybir.dt.float16

    nc = bacc.Bacc("TRN2", target_bir_lowering=False, debug=False, num_devices=N_CORES)

    xt_d = nc.declare_dram_parameter("xt", [K, B_LOCAL], FP16, isOutput=False)
    w_d = nc.declare_dram_parameter("w2", [K, K], FP16, isOutput=False)
    o_d = nc.declare_dram_parameter("out", [K, B_LOCAL], FP16, isOutput=True)

    with tile.TileContext(nc) as tc, ExitStack() as ctx:
        cpool = ctx.enter_context(tc.tile_pool(name="const", bufs=1))
        wpool = ctx.enter_context(tc.tile_pool(name="w", bufs=1))
        xtpool = ctx.enter_context(tc.tile_pool(name="xt", bufs=1))
        ocpool = ctx.enter_context(tc.tile_pool(name="oc", bufs=8))
        pspool = ctx.enter_context(tc.tile_pool(name="ps", bufs=2, space="PSUM"))

        # scratch tile for PE-clock warmup matmuls
        warm = cpool.tile([P, 256], FP16)
        nc.vector.memset(warm[:], 0.0)

        # --- input DMAs ------------------------------------------------
        # ramp-critical stream on the Sync ring: (wa[kt] = w2 cols 0:256,
        # xT[kt]) pairs; the first xT slice rides the Scalar ring in
        # parallel; w2 cols 256:1280 (n-tiles 2..9) stream in behind.
        wa_sb, wb_sb, xt_sb = [], [], []

        wa0 = wpool.tile([P, WA], FP16, name="wa0", tag="wa0")
        nc.sync.dma_start(wa0[:], w_d[0:P, 0:WA])
        wa_sb.append(wa0)
        xt0a = xtpool.tile([P, 512], FP16, name="xt0a", tag="xt0a")
        nc.scalar.dma_start(xt0a[:], xt_d[0:P, 0:512])
        xt0b = xtpool.tile([P, B_LOCAL - 512], FP16, name="xt0b", tag="xt0b")
        nc.sync.dma_start(xt0b[:], xt_d[0:P, 512:B_LOCAL])
        xt_sb.append((xt0a, xt0b))
        for kt in range(1, NKT):
            t = wpool.tile([P, WA], FP16, name=f"wa{kt}", tag=f"wa{kt}")
            nc.sync.dma_start(t[:], w_d[kt * P:(kt + 1) * P, 0:WA])
            wa_sb.append(t)
            t = xtpool.tile([P, B_LOCAL], FP16, name=f"xt{kt}", tag=f"xt{kt}")
            # kt=1 rides the otherwise-idle Scalar ring so the k-tile 1
            # matmuls don't wait behind the Sync ring's early stream
            (nc.scalar if kt == 1 else nc.sync).dma_start(
                t[:], xt_d[kt * P:(kt + 1) * P, :])
            xt_sb.append(t)
        for kt in range(NKT):
            t = wpool.tile([P, K - WA], FP16, name=f"wb{kt}", tag=f"wb{kt}")
            nc.sync.dma_start(t[:], w_d[kt * P:(kt + 1) * P, WA:K])
            wb_sb.append(t)

        def xt_chunk(kt, c):
            sl = slice(c * 512, (c + 1) * 512)
            if kt == 0:
                return xt0a[:] if c == 0 else xt0b[:, (c - 1) * 512:c * 512]
            return xt_sb[kt][:, sl]

        def w_block(kt, nt):
            if nt < 2:
                return wa_sb[kt][:, nt * P:(nt + 1) * P]
            return wb_sb[kt][:, (nt - 2) * P:(nt - 1) * P]

        # per-chunk PSUM tiles: 4 tags x bufs=2 x 1 bank = all 8 banks
        def alloc_ps(nt):
            return [pspool.tile([P, 512], F32, name=f"ps{nt}_{c}", tag=f"ps{c}")
                    for c in range(NCH)]

        ps = {0: alloc_ps(0), 1: alloc_ps(1)}

        # warmup: keep the PE busy (HAM at 2.4GHz) while the first operands
        # are in DMA flight; writes are re-cleared by the first real
        # start=True matmul on that bank.
        for _ in range(N_WARM):
            nc.tensor.matmul(ps[0][0][:, 0:256], warm[:, 0:128], warm[:],
                             start=True, stop=True)

        def emit_mm(nt, kt, c):
            nc.tensor.matmul(ps[nt % 2][c][:], w_block(kt, nt), xt_chunk(kt, c),
                             start=(kt == 0), stop=(kt == NKT - 1))

        def emit_epilogue_chunk(nt, c):
            sl = slice(c * 512, (c + 1) * 512)
            o_c = ocpool.tile([P, 512], FP16, name=f"o{nt}_{c}", tag="oc")
            nc.vector.tensor_mul(o_c[:], xt_chunk(nt, c), ps[nt % 2][c][:])
            nc.sync.dma_start(o_d[nt * P:(nt + 1) * P, sl], o_c[:])

        # ramp: n-tiles 0 and 1 interleaved per k-tile so each arriving
        # (wa, xT) pair unlocks work for both
        for kt in range(NKT):
            for nt in (0, 1):
                for c in range(NCH):
                    emit_mm(nt, kt, c)
        for nt in (0, 1):
            for c in range(NCH):
                emit_epilogue_chunk(nt, c)

        # steady state: k-contiguous per chunk; each chunk's epilogue and
        # output DMA overlap the following chunks' matmuls
        for nt in range(2, NKT):
            ps[nt % 2] = alloc_ps(nt)
            last = nt == NKT - 1
            for c in range(NCH - 1 if last else NCH):
                for kt in range(NKT):
                    emit_mm(nt, kt, c)
                emit_epilogue_chunk(nt, c)

        # very last chunk split in half across two free psum banks so the
        # final epilogue + output DMA trail only ~a quarter-chunk of matmuls
        nt = NKT - 1
        hA, hB = ps[nt % 2][NCH - 1], pspool.tile([P, 256], F32,
                                                  name="ps_lastb", tag="ps0")
        halves = [(hA[:, 0:256], slice(1536, 1792)), (hB[:], slice(1792, 2048))]

        def xt_slice(kt, sl):
            if kt == 0:
                return xt0b[:, sl.start - 512:sl.stop - 512]
            return xt_sb[kt][:, sl]

        for h, sl in halves:
            for kt in range(NKT):
                nc.tensor.matmul(h, w_block(kt, nt), xt_slice(kt, sl),
                                 start=(kt == 0), stop=(kt == NKT - 1))
        for h, sl in halves:
            o_h = ocpool.tile([P, 256], FP16, name=f"oL{sl.start}", tag="oc2")
            nc.vector.tensor_mul(o_h[:], xt_sb[nt][:, sl], h)
            nc.sync.dma_start(o_d[nt * P:(nt + 1) * P, sl], o_h[:])

    nc.compile()
    return nc


def _get_nc():
    if "nc" not in _NC_CACHE:
        _NC_CACHE["nc"] = _build_kernel()
    return _NC_CACHE["nc"]


def _ensure_axon_hooks():
    """bass_utils imports antenv.axon_hooks when BASS_TRACE is set; register
    the real ctypes-based NTFF hook if available, else a no-op registry so
    tracing degrades gracefully instead of crashing."""
    try:
        import antenv.axon_hooks  # noqa: F401
    except ImportError:
        import sys
        import types

        m = types.ModuleType("antenv.axon_hooks")
        m._HOOK = None
        m.set_axon_ntff_profile_hook = lambda h: setattr(m, "_HOOK", h)
        m.get_axon_ntff_profile_hook = lambda: m._HOOK
        sys.modules["antenv.axon_hooks"] = m
        try:
            from trn_agent_boot.trn_boot import _ntff_profile_via_ctypes

            m._HOOK = _ntff_profile_via_ctypes("/opt/axon/libaxon_pjrt.so")
        except Exception:
            pass


def _make_in_maps(embeddings: np.ndarray, bilinear_W: np.ndarray):
    F, D = NUM_FIELDS, EMBED_DIM
    # fold the f!=g mask into W and permute to the GEMM layout:
    # w2[g*D+e, f*D+d] = W[f,g,d,e] * (f != g)
    mask = (1.0 - np.eye(F, dtype=np.float32))[:, :, None, None]
    w2 = np.ascontiguousarray(
        (np.asarray(bilinear_W, dtype=np.float32) * mask)
        .transpose(1, 3, 0, 2).reshape(F * D, F * D)).astype(np.float16)

    x = np.asarray(embeddings, dtype=np.float32).reshape(BATCH, K).astype(np.float16)
    in_maps = []
    for c in range(N_CORES):
        xt = np.ascontiguousarray(x[c * B_LOCAL:(c + 1) * B_LOCAL, :].T)
        in_maps.append({"xt": xt, "w2": w2})
    return in_maps


def kernel(embeddings: np.ndarray, bilinear_W: np.ndarray) -> np.ndarray:
    _ensure_axon_hooks()
    from concourse.bass_utils import run_bass_kernel_spmd

    in_maps = _make_in_maps(embeddings, bilinear_W)
    nc = _get_nc()
    res = run_bass_kernel_spmd(nc, in_maps, list(range(N_CORES)))
    out = np.concatenate(
        [res.results[i]["out"].T for i in range(N_CORES)], axis=0)
    return np.ascontiguousarray(out).astype(np.float32).reshape(
        BATCH, NUM_FIELDS, EMBED_DIM)
